# revision 22
# baseline (speedup 1.0000x reference)
"""Bass/Tile TRN2 kernel for nn_MultiHeadAttention_85538568667794.

Disentangled (DeBERTa-style) multi-head attention with a causal sliding
window (256), ALiBi bias, returning both the projected output and the
full (B,H,T,T) attention-probability tensor.

Sharding: 8 cores = (batch b in 0..3) x (sequence half in 0..1).
Each core handles ALL 16 heads for its 512 query rows, computing K/V over
a 768-row window (512 own rows + 256-row halo). Every output row is then
complete locally -> no cross-core communication at all.

Score math:  S = (q . (k/8 + pk) + pq . k) / sqrt(192)
implemented as two K=64 matmuls accumulated in one PSUM tile, with the
band mask + ALiBi folded into one additive (128,384) bias strip per
(qt, head) that is precomputed on the host (data-only per-core variation
handles the left sequence edge, keeping the program uniform across cores).

Softmax: exp on ScalarE with accum_out giving the row sum for free (logits
are bounded, so no max-subtraction is needed; masked entries are -1e30 and
underflow to exactly 0, matching the reference's -inf semantics).

wei is written to DRAM band-strips only; ExternalOutput buffers are
pre-zeroed by the runtime (documented behavior relied on by kernels), so
out-of-band entries are exactly 0.
"""

import math
import sys

import numpy as np

sys.path.insert(0, "/opt/trn_rl_repo")

import ml_dtypes  # noqa: E402

import concourse.bass as bass  # noqa: E402
import concourse.mybir as mybir  # noqa: E402
import concourse.tile as tile  # noqa: E402
from concourse import bacc, bass_utils  # noqa: E402
from concourse.masks import make_identity  # noqa: E402

F32 = mybir.dt.float32
BF16 = mybir.dt.bfloat16
NPBF16 = ml_dtypes.bfloat16

B, T, C = 4, 1024, 1024
H, HD = 16, 64
WIN = 256
NCORES = 8
ROWS = 512  # query rows per core
KV = 768  # k/v window rows per core (256 halo + 512 own)
S192 = 1.0 / math.sqrt(3 * HD)
NEG = -1.0e30
P = 128

Exp = mybir.ActivationFunctionType.Exp
Copy = mybir.ActivationFunctionType.Copy
Ident = mybir.ActivationFunctionType.Identity
ADD = mybir.AluOpType.add
MULT = mybir.AluOpType.mult


def _emit(nc, tc, io):
    """Emit the uniform per-core program."""
    from contextlib import ExitStack

    with ExitStack() as ctx:
        const = ctx.enter_context(tc.tile_pool(name="const", bufs=1))
        ident = const.tile([P, P], BF16, tag="ident")
        make_identity(nc, ident[:])
        bQ = const.tile([P, 8], F32, tag="bQ")
        bPQ = const.tile([P, 8], F32, tag="bPQ")
        bK = const.tile([P, 8], F32, tag="bK")
        bKP = const.tile([P, 8], F32, tag="bKP")
        bO = const.tile([P, 8], F32, tag="bO")
        for t_, n_ in ((bQ, "bQ"), (bPQ, "bPQ"), (bK, "bK"), (bKP, "bKP"), (bO, "bO")):
            nc.sync.dma_start(t_[:], io[n_][:])

        big = ctx.enter_context(tc.tile_pool(name="big", bufs=1))
        xts = big.tile([P, 8, KV], BF16, tag="xts")
        pts = big.tile([P, 8, KV], BF16, tag="pts")
        # per-kt xts loads are interleaved into the Wq stream below
        # resident weights (used in multi-pass loops); DMAs are emitted
        # after the q/pq projections so the first matmuls aren't queued
        # behind 8MB of weight loads.
        wk8 = big.tile([P, 8, 1024], BF16, tag="wk8")
        wpk = big.tile([P, 8, 1024], BF16, tag="wpk")
        wv = big.tile([P, 8, 1024], BF16, tag="wv")

        # per-head interleaved score operands, built from the projection
        # psums via SBUF->SBUF DMA shuffles (DMA moves across partitions):
        #   QC[:, h] = [q_h * s ; pq_h * s]   (contraction dim 128)
        #   KC[:, h] = [k'_h    ; k_h     ]   with k' = k/8 + pk
        QC = big.tile([P, H, ROWS], BF16, tag="QC")
        KC = big.tile([P, H, KV], BF16, tag="KC")
        VS = big.tile([P, 6, 1024], BF16, tag="VS")  # v, natural (rows, dv)
        OUTC = big.tile([P, 8, ROWS], BF16, tag="OUTC")  # attn out^T (c, m)

        wpool = ctx.enter_context(tc.tile_pool(name="wpool", bufs=4))

        # ---------------- projections ----------------
        with (
            tc.tile_pool(name="pjps", bufs=8, space="PSUM") as pjps,
            tc.tile_pool(name="stg", bufs=3) as stg,
        ):
            # q over own rows (xts cols 256:768); staging tiles hold heads
            # (2t, 2t+1) in psum partition layout; SBUF->SBUF DMAs
            # interleave 64-row halves into QC per head.
            # QC rows 64:128 = q*s ; rows 0:64 = (q + 8*pq)*s.
            qsb = []
            ps = [pjps.tile([P, ROWS], F32, tag="pj", name="pj") for _ in range(8)]
            for kt in range(8):
                nc.sync.dma_start(xts[:, kt, :], io["xT"][kt])
                w = wpool.tile([P, 1024], BF16, tag="w")
                nc.sync.dma_start(w[:], io["Wq"][kt])
                for t in range(8):
                    nc.tensor.matmul(
                        ps[t][:],
                        lhsT=w[:, t * P : (t + 1) * P],
                        rhs=xts[:, kt, 256:768],
                        start=(kt == 0),
                        stop=(kt == 7),
                    )
            for t in range(8):
                sq = stg.tile([P, ROWS], BF16, tag="sq", name="sq", bufs=8)
                nc.vector.tensor_scalar(
                    sq[:], ps[t][:],
                    scalar1=S192, scalar2=bQ[:, t : t + 1],
                    op0=MULT, op1=ADD,
                )
                qsb.append(sq)
                nc.scalar.dma_start(QC[64:P, 2 * t, :], sq[0:64, :])
                nc.scalar.dma_start(QC[64:P, 2 * t + 1, :], sq[64:P, :])
            for kt in range(8):
                nc.sync.dma_start(pts[:, kt, :], io["pT"][kt])
            ps = [pjps.tile([P, ROWS], F32, tag="pj", name="pj") for _ in range(8)]
            for kt in range(8):
                w = wpool.tile([P, 1024], BF16, tag="w")
                nc.sync.dma_start(w[:], io["Wpq"][kt])
                for t in range(8):
                    nc.tensor.matmul(
                        ps[t][:],
                        lhsT=w[:, t * P : (t + 1) * P],
                        rhs=pts[:, kt, 256:768],
                        start=(kt == 0),
                        stop=(kt == 7),
                    )
            for t in range(8):
                s2 = stg.tile([P, ROWS], BF16, tag="s2", name="s2")
                nc.vector.tensor_scalar(
                    s2[:], ps[t][:],
                    scalar1=8.0 * S192, scalar2=bPQ[:, t : t + 1],
                    op0=MULT, op1=ADD,
                )
                sq2 = stg.tile([P, ROWS], BF16, tag="sq2", name="sq2")
                nc.vector.tensor_tensor(sq2[:], s2[:], qsb[t][:], op=ADD)
                nc.scalar.dma_start(QC[0:64, 2 * t, :], sq2[0:64, :])
                nc.scalar.dma_start(QC[0:64, 2 * t + 1, :], sq2[64:P, :])
            for kt in range(8):
                nc.sync.dma_start(wk8[:, kt, :], io["Wk8"][kt])
                nc.sync.dma_start(wpk[:, kt, :], io["Wpk"][kt])
                nc.sync.dma_start(wv[:, kt, :], io["Wv"][kt])
            # k/8 -> KC rows 0:64 ; pk -> KC rows 64:128, over the KV window
            for wres, rt, krow, bias_t in (
                (wk8, xts, 0, bK),
                (wpk, pts, 64, bKP),
            ):
                for ch in range(2):
                    sl = slice(ch * 384, ch * 384 + 384)
                    ps = [
                        pjps.tile([P, 384], F32, tag="pj", name="pj")
                        for _ in range(8)
                    ]
                    for kt in range(8):
                        for t in range(8):
                            nc.tensor.matmul(
                                ps[t][:],
                                lhsT=wres[:, kt, t * P : (t + 1) * P],
                                rhs=rt[:, kt, sl],
                                start=(kt == 0),
                                stop=(kt == 7),
                            )
                    for t in range(8):
                        sk = stg.tile([P, 384], BF16, tag="sk", name="sk")
                        nc.scalar.activation(
                            sk[:], ps[t][:], Ident, bias=bias_t[:, t : t + 1]
                        )
                        nc.scalar.dma_start(
                            KC[krow : krow + 64, 2 * t, sl], sk[0:64, :]
                        )
                        nc.scalar.dma_start(
                            KC[krow : krow + 64, 2 * t + 1, sl], sk[64:P, :]
                        )
            # v, natural orientation (kv rows, dv); bias bv folded into bO
            for ch in range(2):
                sl = slice(ch * 512, ch * 512 + 512)
                ps = [pjps.tile([P, 512], F32, tag="pj", name="pj") for _ in range(6)]
                for kt in range(8):
                    for mt in range(6):
                        nc.tensor.matmul(
                            ps[mt][:],
                            lhsT=xts[:, kt, mt * P : (mt + 1) * P],
                            rhs=wv[:, kt, sl],
                            start=(kt == 0),
                            stop=(kt == 7),
                        )
                for mt in range(6):
                    nc.any.tensor_copy(VS[:, mt, sl], ps[mt][:])

        # ---------------- attention ----------------
        with (
            tc.tile_pool(name="attps", bufs=4, space="PSUM") as attps,
            tc.tile_pool(name="ptps", bufs=2, space="PSUM") as ptps,
            tc.tile_pool(name="ops", bufs=2, space="PSUM") as ops,
            tc.tile_pool(name="attsb", bufs=6) as attsb,
            tc.tile_pool(name="ptsb", bufs=6) as ptsb,
        ):
            # software-pipelined by one iteration: each tile's post-exp
            # tail (recip/normalize/transposes/PV) is emitted one
            # iteration later so DVE's in-order stream never has the
            # next tile's bias-add queued behind an ACT round-trip.
            def emit_tail(st):
                h, qt, hp, po, pf, rr, outps = st
                ri = attsb.tile([P, 1], F32, tag="ri", name="ri")
                nc.vector.reciprocal(ri[:], rr[:])
                pn = attsb.tile([P, 384], F32, tag="pn", name="pn")
                nc.vector.tensor_scalar_mul(pn[:], pf[:], ri[:])
                nc.gpsimd.dma_start(
                    io["weiB"][h, qt, :, qt * P : qt * P + 384], pn[:]
                )
                pnb = attsb.tile([P, 384], BF16, tag="pnb", name="pnb")
                nc.scalar.activation(pnb[:], pf[:], Copy, scale=ri[:])
                ptp = ptps.tile([P, 3, P], BF16, tag="pt", name="pt")
                for c_ in range(3):
                    nc.tensor.transpose(
                        ptp[:, c_, :], pnb[:, c_ * P : (c_ + 1) * P], ident[:]
                    )
                pts_ = ptsb.tile([P, 384], BF16, tag="ptsb", name="ptsb")
                nc.any.tensor_copy(pts_[:], ptp[:])
                for c_ in range(3):
                    nc.tensor.matmul(
                        outps[po : po + 64, qt, :],
                        lhsT=VS[:, qt + c_, h * 64 : (h + 1) * 64],
                        rhs=pts_[:, c_ * P : (c_ + 1) * P],
                        start=(c_ == 0),
                        stop=(c_ == 2),
                    )
                if po == 64 and qt == 3:
                    for q2 in range(4):
                        nc.any.tensor_copy(
                            OUTC[:, hp, q2 * P : (q2 + 1) * P], outps[:, q2, :]
                        )

            pend = []
            for hp in range(8):
                # all 4 qt accumulators packed in one PSUM bank
                outps = ops.tile([P, 4, P], F32, tag="o", name="o")
                for hs in range(2):
                    h = 2 * hp + hs
                    po = 64 * hs
                    for qt in range(4):
                        j0 = qt * P  # strip start in local kv coords
                        sps = attps.tile([P, 384], F32, tag="s", name="s")
                        nc.tensor.matmul(
                            sps[:],
                            lhsT=QC[:, h, qt * P : (qt + 1) * P],
                            rhs=KC[:, h, j0 : j0 + 384],
                            start=True,
                            stop=True,
                        )
                        bia = attsb.tile([P, 384], BF16, tag="bias", name="bias")
                        nc.sync.dma_start(bia[:], io["biasS"][qt, h])
                        ssb = attsb.tile([P, 384], F32, tag="ssb", name="ssb")
                        nc.vector.tensor_tensor(ssb[:], sps[:], bia[:], op=ADD)
                        pf = attsb.tile([P, 384], F32, tag="pf", name="pf")
                        rr = attsb.tile([P, 1], F32, tag="rr", name="rr")
                        nc.scalar.activation(pf[:], ssb[:], Exp, accum_out=rr[:])
                        pend.append((h, qt, hp, po, pf, rr, outps))
                        if len(pend) > 2:
                            emit_tail(pend.pop(0))
            for st in pend:
                emit_tail(st)

        # ---------------- output projection (transposed) ----------------
        with (
            tc.tile_pool(name="fps", bufs=8, space="PSUM") as fps,
            tc.tile_pool(name="fsb", bufs=2) as fsb,
        ):
            Fps = [fps.tile([P, ROWS], F32, tag="f", name="f") for _ in range(8)]
            for ct in range(8):
                w = wpool.tile([P, 1024], BF16, tag="w")
                nc.sync.dma_start(w[:], io["Wo"][ct])
                for nt in range(8):
                    nc.tensor.matmul(
                        Fps[nt][:],
                        lhsT=w[:, nt * P : (nt + 1) * P],
                        rhs=OUTC[:, ct, :],
                        start=(ct == 0),
                        stop=(ct == 7),
                    )
            for nt in range(8):
                fo = fsb.tile([P, ROWS], F32, tag="fo")
                nc.scalar.activation(
                    fo[:], Fps[nt][:], Ident, bias=bO[:, nt : nt + 1]
                )
                nc.scalar.dma_start(io["foutT"][nt], fo[:])


_CACHE = {}


def _build():
    if "nc" in _CACHE:
        return _CACHE["nc"], _CACHE["io"]
    nc = bacc.Bacc(
        "TRN2",
        target_bir_lowering=False,
        debug=False,
        enable_asserts=False,
        num_devices=NCORES,
    )
    io = {}
    io["xT"] = nc.dram_tensor("xT", (8, P, KV), BF16, kind="ExternalInput").ap()
    io["pT"] = nc.dram_tensor("pT", (8, P, KV), BF16, kind="ExternalInput").ap()
    for wn in ("Wq", "Wpq", "Wk8", "Wpk", "Wv", "Wo"):
        io[wn] = nc.dram_tensor(wn, (8, P, 1024), BF16, kind="ExternalInput").ap()
    for bn in ("bQ", "bPQ", "bK", "bKP", "bO"):
        io[bn] = nc.dram_tensor(bn, (P, 8), F32, kind="ExternalInput").ap()
    io["biasS"] = nc.dram_tensor(
        "biasS", (4, H, P, 384), BF16, kind="ExternalInput"
    ).ap()
    io["weiB"] = nc.dram_tensor(
        "weiB", (H, 4, P, KV), F32, kind="ExternalOutput"
    ).ap()
    io["foutT"] = nc.dram_tensor(
        "foutT", (8, P, ROWS), F32, kind="ExternalOutput"
    ).ap()

    with tile.TileContext(nc) as tc:
        _emit(nc, tc, io)
    nc.compile()
    _CACHE["nc"] = nc
    _CACHE["io"] = io
    return nc, io


def _host_prep(x, pos_emb, Wq, bq, Wk, bk, Wv, bv, Wpq, bpq, Wpk, bpk, Wo, bo):
    """Build per-core input maps (host-side data formatting only)."""
    x = np.asarray(x, np.float32)
    pos_emb = np.asarray(pos_emb, np.float32)
    weights = {
        "Wq": np.asarray(Wq, np.float32),
        "Wpq": np.asarray(Wpq, np.float32),
        "Wk": np.asarray(Wk, np.float32),
        "Wpk": np.asarray(Wpk, np.float32),
        "Wv": np.asarray(Wv, np.float32),
        "Wo": np.asarray(Wo, np.float32),
    }
    weights["Wk8"] = weights["Wk"] * 0.125

    shared = {}
    for wn, wv_ in weights.items():
        if wn == "Wk":
            continue  # only Wk8 = Wk/8 is shipped
        shared[wn] = np.ascontiguousarray(
            wv_.astype(NPBF16).reshape(8, P, 1024)
        )

    def bcol(v):  # (1024,) -> (128, 8) with column t = v[128t:128(t+1)]
        return np.ascontiguousarray(
            np.asarray(v, np.float32).reshape(8, P).T
        ).astype(np.float32)

    shared["bQ"] = bcol(np.asarray(bq, np.float32) * S192)
    shared["bPQ"] = bcol(np.asarray(bpq, np.float32) * 8.0 * S192)
    shared["bK"] = bcol(np.asarray(bk, np.float32) * 0.125)
    shared["bKP"] = bcol(np.asarray(bpk, np.float32))
    shared["bO"] = bcol(
        np.asarray(bo, np.float64)
        + np.asarray(bv, np.float64) @ np.asarray(Wo, np.float64)
    )

    # additive bias strips: rel = r - s + 256 ; allowed iff 0 <= rel <= 256
    slopes = 2.0 ** (-(8.0 / H) * np.arange(1, H + 1, dtype=np.float64))
    r = np.arange(P)[:, None]
    s = np.arange(384)[None, :]
    rel = r - s + 256
    allowed = (rel >= 0) & (rel <= 256)
    base = np.where(
        allowed[None, :, :],
        -slopes[:, None, None] * rel[None, :, :],
        NEG,
    ).astype(np.float32)  # (H, 128, 384)
    bias_int = np.broadcast_to(base, (4, H, P, 384)).copy()  # interior cores
    bias_edge = bias_int.copy()  # half==0: mask phantom j<0 (qt*128 + s < 256)
    for qt in range(4):
        ph = (qt * P + np.arange(384)) < 256
        if ph.any():
            bias_edge[qt, :, :, ph] = NEG
    bias_int = bias_int.astype(NPBF16)
    bias_edge = bias_edge.astype(NPBF16)

    in_maps = []
    for core in range(NCORES):
        b, half = divmod(core, 2)
        m = dict(shared)
        xt = np.zeros((C, KV), NPBF16)
        pt = np.zeros((C, KV), NPBF16)
        if half == 0:
            xt[:, 256:] = x[b].T[:, 0:ROWS].astype(NPBF16)
            pt[:, 256:] = pos_emb[b].T[:, 0:ROWS].astype(NPBF16)
            m["biasS"] = bias_edge
        else:
            xt[:] = x[b].T[:, 256:1024].astype(NPBF16)
            pt[:] = pos_emb[b].T[:, 256:1024].astype(NPBF16)
            m["biasS"] = bias_int
        m["xT"] = np.ascontiguousarray(xt.reshape(8, P, KV))
        m["pT"] = np.ascontiguousarray(pt.reshape(8, P, KV))
        in_maps.append(m)
    return in_maps


def run(inputs, trace=False, **kw):
    nc, io = _build()
    in_maps = _host_prep(**inputs)
    res = bass_utils.run_bass_kernel_spmd(
        nc, in_maps, core_ids=list(range(NCORES)), trace=trace, **kw
    )
    out = np.empty((B, T, C), np.float32)
    wei = np.zeros((B, H, T, T), np.float32)
    for core in range(NCORES):
        b, half = divmod(core, 2)
        r = res.results[core]
        foutT = np.asarray(r["foutT"], np.float32).reshape(C, ROWS)
        weiB = np.asarray(r["weiB"], np.float32).reshape(H, ROWS, KV)
        out[b, half * ROWS : (half + 1) * ROWS, :] = foutT.T
        if half == 0:
            wei[b, :, 0:ROWS, 0:ROWS] = weiB[:, :, 256:KV]
        else:
            wei[b, :, ROWS:T, 256:T] = weiB
    return (out, wei), res


def kernel(**inputs):
    outs, _ = run(inputs, trace=False)
    return outs


# revision 23
# speedup vs baseline: 1.0149x; 1.0149x over previous
"""Bass/Tile TRN2 kernel for nn_MultiHeadAttention_85538568667794.

Disentangled (DeBERTa-style) multi-head attention with a causal sliding
window (256), ALiBi bias, returning both the projected output and the
full (B,H,T,T) attention-probability tensor.

Sharding: 8 cores = (batch b in 0..3) x (sequence half in 0..1).
Each core handles ALL 16 heads for its 512 query rows, computing K/V over
a 768-row window (512 own rows + 256-row halo). Every output row is then
complete locally -> no cross-core communication at all.

Score math:  S = (q.k/8 + q.pk + pq.k) / sqrt(192)
           = [(q + 8 pq) s ; q s] . [k/8 ; pk]      (s = 1/sqrt(192))
i.e. a single K=128 matmul per (head, qtile) over partition-interleaved
operands QC/KC built by SBUF->SBUF DMA shuffles (only DMA can move data
across partitions); this also removes the raw-k projection chain.
The band mask + ALiBi are folded into one additive (128,384) bias strip
per (qt, head), precomputed on the host (data-only per-core variation
handles the left sequence edge, keeping the program uniform across
cores). The attention loop is software-pipelined by one iteration so the
in-order DVE stream never queues the next tile's bias-add behind an
ACT round-trip.

Softmax: exp on ScalarE with accum_out giving the row sum for free (logits
are bounded, so no max-subtraction is needed; masked entries are -1e30 and
underflow to exactly 0, matching the reference's -inf semantics).

wei is written to DRAM band-strips only; ExternalOutput buffers are
pre-zeroed by the runtime (documented behavior relied on by kernels), so
out-of-band entries are exactly 0.
"""

import math
import sys

import numpy as np

sys.path.insert(0, "/opt/trn_rl_repo")

import ml_dtypes  # noqa: E402

import concourse.bass as bass  # noqa: E402
import concourse.mybir as mybir  # noqa: E402
import concourse.tile as tile  # noqa: E402
from concourse import bacc, bass_utils  # noqa: E402
from concourse.masks import make_identity  # noqa: E402

F32 = mybir.dt.float32
BF16 = mybir.dt.bfloat16
NPBF16 = ml_dtypes.bfloat16

B, T, C = 4, 1024, 1024
H, HD = 16, 64
WIN = 256
NCORES = 8
ROWS = 512  # query rows per core
KV = 768  # k/v window rows per core (256 halo + 512 own)
S192 = 1.0 / math.sqrt(3 * HD)
NEG = -1.0e30
P = 128

Exp = mybir.ActivationFunctionType.Exp
Copy = mybir.ActivationFunctionType.Copy
Ident = mybir.ActivationFunctionType.Identity
ADD = mybir.AluOpType.add
MULT = mybir.AluOpType.mult


def _emit(nc, tc, io):
    """Emit the uniform per-core program."""
    from contextlib import ExitStack

    with ExitStack() as ctx:
        const = ctx.enter_context(tc.tile_pool(name="const", bufs=1))
        ident = const.tile([P, P], BF16, tag="ident")
        make_identity(nc, ident[:])
        bQ = const.tile([P, 8], F32, tag="bQ")
        bPQ = const.tile([P, 8], F32, tag="bPQ")
        bK = const.tile([P, 8], F32, tag="bK")
        bKP = const.tile([P, 8], F32, tag="bKP")
        bO = const.tile([P, 8], F32, tag="bO")
        for t_, n_ in ((bQ, "bQ"), (bPQ, "bPQ"), (bK, "bK"), (bKP, "bKP"), (bO, "bO")):
            nc.sync.dma_start(t_[:], io[n_][:])

        big = ctx.enter_context(tc.tile_pool(name="big", bufs=1))
        xts = big.tile([P, 8, KV], BF16, tag="xts")
        pts = big.tile([P, 8, KV], BF16, tag="pts")
        # per-kt xts loads are interleaved into the Wq stream below
        # resident weights (used in multi-pass loops); DMAs are emitted
        # after the q/pq projections so the first matmuls aren't queued
        # behind 8MB of weight loads.
        wk8 = big.tile([P, 8, 1024], BF16, tag="wk8")
        wpk = big.tile([P, 8, 1024], BF16, tag="wpk")
        wv = big.tile([P, 8, 1024], BF16, tag="wv")

        # per-head interleaved score operands, built from the projection
        # psums via SBUF->SBUF DMA shuffles (DMA moves across partitions):
        #   QC[:, h] = [q_h * s ; pq_h * s]   (contraction dim 128)
        #   KC[:, h] = [k'_h    ; k_h     ]   with k' = k/8 + pk
        QC = big.tile([P, H, ROWS], BF16, tag="QC")
        KC = big.tile([P, H, KV], BF16, tag="KC")
        VS = big.tile([P, 6, 1024], BF16, tag="VS")  # v, natural (rows, dv)
        OUTC = big.tile([P, 8, ROWS], BF16, tag="OUTC")  # attn out^T (c, m)

        wpool = ctx.enter_context(tc.tile_pool(name="wpool", bufs=4))

        # ---------------- projections ----------------
        with (
            tc.tile_pool(name="pjps", bufs=8, space="PSUM") as pjps,
            tc.tile_pool(name="stg", bufs=3) as stg,
        ):
            # q over own rows (xts cols 256:768); staging tiles hold heads
            # (2t, 2t+1) in psum partition layout; SBUF->SBUF DMAs
            # interleave 64-row halves into QC per head.
            # QC rows 64:128 = q*s ; rows 0:64 = (q + 8*pq)*s.
            qsb = []
            ps = [pjps.tile([P, ROWS], F32, tag="pj", name="pj") for _ in range(8)]
            for kt in range(8):
                nc.sync.dma_start(xts[:, kt, :], io["xT"][kt])
                w = wpool.tile([P, 1024], BF16, tag="w")
                nc.sync.dma_start(w[:], io["Wq"][kt])
                for t in range(8):
                    nc.tensor.matmul(
                        ps[t][:],
                        lhsT=w[:, t * P : (t + 1) * P],
                        rhs=xts[:, kt, 256:768],
                        start=(kt == 0),
                        stop=(kt == 7),
                    )
            for t in range(8):
                sq = stg.tile([P, ROWS], BF16, tag="sq", name="sq", bufs=8)
                nc.vector.tensor_scalar(
                    sq[:], ps[t][:],
                    scalar1=S192, scalar2=bQ[:, t : t + 1],
                    op0=MULT, op1=ADD,
                )
                qsb.append(sq)
                nc.scalar.dma_start(QC[64:P, 2 * t, :], sq[0:64, :])
                nc.scalar.dma_start(QC[64:P, 2 * t + 1, :], sq[64:P, :])
            for kt in range(8):
                nc.sync.dma_start(pts[:, kt, :], io["pT"][kt])
            ps = [pjps.tile([P, ROWS], F32, tag="pj", name="pj") for _ in range(8)]
            for kt in range(8):
                w = wpool.tile([P, 1024], BF16, tag="w")
                nc.sync.dma_start(w[:], io["Wpq"][kt])
                for t in range(8):
                    nc.tensor.matmul(
                        ps[t][:],
                        lhsT=w[:, t * P : (t + 1) * P],
                        rhs=pts[:, kt, 256:768],
                        start=(kt == 0),
                        stop=(kt == 7),
                    )
            for t in range(8):
                s2 = stg.tile([P, ROWS], BF16, tag="s2", name="s2")
                nc.vector.tensor_scalar(
                    s2[:], ps[t][:],
                    scalar1=8.0 * S192, scalar2=bPQ[:, t : t + 1],
                    op0=MULT, op1=ADD,
                )
                sq2 = stg.tile([P, ROWS], BF16, tag="sq2", name="sq2")
                nc.vector.tensor_tensor(sq2[:], s2[:], qsb[t][:], op=ADD)
                nc.scalar.dma_start(QC[0:64, 2 * t, :], sq2[0:64, :])
                nc.scalar.dma_start(QC[0:64, 2 * t + 1, :], sq2[64:P, :])
            for kt in range(8):
                nc.sync.dma_start(wk8[:, kt, :], io["Wk8"][kt])
                nc.sync.dma_start(wpk[:, kt, :], io["Wpk"][kt])
                nc.sync.dma_start(wv[:, kt, :], io["Wv"][kt])
            # k/8 -> KC rows 0:64 ; pk -> KC rows 64:128, over the KV window
            for wres, rt, krow, bias_t in (
                (wk8, xts, 0, bK),
                (wpk, pts, 64, bKP),
            ):
                for ch in range(2):
                    sl = slice(ch * 384, ch * 384 + 384)
                    ps = [
                        pjps.tile([P, 384], F32, tag="pj", name="pj")
                        for _ in range(8)
                    ]
                    for kt in range(8):
                        for t in range(8):
                            nc.tensor.matmul(
                                ps[t][:],
                                lhsT=wres[:, kt, t * P : (t + 1) * P],
                                rhs=rt[:, kt, sl],
                                start=(kt == 0),
                                stop=(kt == 7),
                            )
                    for t in range(8):
                        sk = stg.tile([P, 384], BF16, tag="sk", name="sk")
                        nc.scalar.activation(
                            sk[:], ps[t][:], Ident, bias=bias_t[:, t : t + 1]
                        )
                        nc.scalar.dma_start(
                            KC[krow : krow + 64, 2 * t, sl], sk[0:64, :]
                        )
                        nc.scalar.dma_start(
                            KC[krow : krow + 64, 2 * t + 1, sl], sk[64:P, :]
                        )
            # v, natural orientation (kv rows, dv); bias bv folded into bO
            for ch in range(2):
                sl = slice(ch * 512, ch * 512 + 512)
                ps = [pjps.tile([P, 512], F32, tag="pj", name="pj") for _ in range(6)]
                for kt in range(8):
                    for mt in range(6):
                        nc.tensor.matmul(
                            ps[mt][:],
                            lhsT=xts[:, kt, mt * P : (mt + 1) * P],
                            rhs=wv[:, kt, sl],
                            start=(kt == 0),
                            stop=(kt == 7),
                        )
                for mt in range(6):
                    nc.any.tensor_copy(VS[:, mt, sl], ps[mt][:])

        # ---------------- attention ----------------
        with (
            tc.tile_pool(name="attps", bufs=4, space="PSUM") as attps,
            tc.tile_pool(name="ptps", bufs=2, space="PSUM") as ptps,
            tc.tile_pool(name="ops", bufs=2, space="PSUM") as ops,
            tc.tile_pool(name="attsb", bufs=6) as attsb,
            tc.tile_pool(name="ptsb", bufs=6) as ptsb,
        ):
            # software-pipelined by one iteration: each tile's post-exp
            # tail (recip/normalize/transposes/PV) is emitted one
            # iteration later so DVE's in-order stream never has the
            # next tile's bias-add queued behind an ACT round-trip.
            def emit_tail(st):
                h, qt, hp, po, pf, rr, outps = st
                ri = attsb.tile([P, 1], F32, tag="ri", name="ri")
                nc.vector.reciprocal(ri[:], rr[:])
                pn = attsb.tile([P, 384], F32, tag="pn", name="pn")
                nc.vector.tensor_scalar_mul(pn[:], pf[:], ri[:])
                nc.gpsimd.dma_start(
                    io["weiB"][h, qt, :, qt * P : qt * P + 384], pn[:]
                )
                pnb = attsb.tile([P, 384], BF16, tag="pnb", name="pnb")
                nc.scalar.activation(pnb[:], pf[:], Copy, scale=ri[:])
                ptp = ptps.tile([P, 3, P], BF16, tag="pt", name="pt")
                for c_ in range(3):
                    nc.tensor.transpose(
                        ptp[:, c_, :], pnb[:, c_ * P : (c_ + 1) * P], ident[:]
                    )
                pts_ = ptsb.tile([P, 384], BF16, tag="ptsb", name="ptsb")
                nc.any.tensor_copy(pts_[:], ptp[:])
                for c_ in range(3):
                    nc.tensor.matmul(
                        outps[po : po + 64, qt, :],
                        lhsT=VS[:, qt + c_, h * 64 : (h + 1) * 64],
                        rhs=pts_[:, c_ * P : (c_ + 1) * P],
                        start=(c_ == 0),
                        stop=(c_ == 2),
                    )
                if po == 64 and qt == 3:
                    for q2 in range(4):
                        nc.any.tensor_copy(
                            OUTC[:, hp, q2 * P : (q2 + 1) * P], outps[:, q2, :]
                        )

            prev = None
            for hp in range(8):
                # all 4 qt accumulators packed in one PSUM bank
                outps = ops.tile([P, 4, P], F32, tag="o", name="o")
                for hs in range(2):
                    h = 2 * hp + hs
                    po = 64 * hs
                    for qt in range(4):
                        j0 = qt * P  # strip start in local kv coords
                        sps = attps.tile([P, 384], F32, tag="s", name="s")
                        nc.tensor.matmul(
                            sps[:],
                            lhsT=QC[:, h, qt * P : (qt + 1) * P],
                            rhs=KC[:, h, j0 : j0 + 384],
                            start=True,
                            stop=True,
                        )
                        bia = attsb.tile([P, 384], BF16, tag="bias", name="bias")
                        nc.sync.dma_start(bia[:], io["biasS"][qt, h])
                        ssb = attsb.tile([P, 384], F32, tag="ssb", name="ssb")
                        nc.vector.tensor_tensor(ssb[:], sps[:], bia[:], op=ADD)
                        pf = attsb.tile([P, 384], F32, tag="pf", name="pf")
                        rr = attsb.tile([P, 1], F32, tag="rr", name="rr")
                        nc.scalar.activation(pf[:], ssb[:], Exp, accum_out=rr[:])
                        if prev is not None:
                            emit_tail(prev)
                        prev = (h, qt, hp, po, pf, rr, outps)
            emit_tail(prev)

        # ---------------- output projection (transposed) ----------------
        with (
            tc.tile_pool(name="fps", bufs=8, space="PSUM") as fps,
            tc.tile_pool(name="fsb", bufs=2) as fsb,
        ):
            Fps = [fps.tile([P, ROWS], F32, tag="f", name="f") for _ in range(8)]
            for ct in range(8):
                w = wpool.tile([P, 1024], BF16, tag="w")
                nc.sync.dma_start(w[:], io["Wo"][ct])
                for nt in range(8):
                    nc.tensor.matmul(
                        Fps[nt][:],
                        lhsT=w[:, nt * P : (nt + 1) * P],
                        rhs=OUTC[:, ct, :],
                        start=(ct == 0),
                        stop=(ct == 7),
                    )
            for nt in range(8):
                fo = fsb.tile([P, ROWS], F32, tag="fo")
                nc.scalar.activation(
                    fo[:], Fps[nt][:], Ident, bias=bO[:, nt : nt + 1]
                )
                nc.scalar.dma_start(io["foutT"][nt], fo[:])


_CACHE = {}


def _build():
    if "nc" in _CACHE:
        return _CACHE["nc"], _CACHE["io"]
    nc = bacc.Bacc(
        "TRN2",
        target_bir_lowering=False,
        debug=False,
        enable_asserts=False,
        num_devices=NCORES,
    )
    io = {}
    io["xT"] = nc.dram_tensor("xT", (8, P, KV), BF16, kind="ExternalInput").ap()
    io["pT"] = nc.dram_tensor("pT", (8, P, KV), BF16, kind="ExternalInput").ap()
    for wn in ("Wq", "Wpq", "Wk8", "Wpk", "Wv", "Wo"):
        io[wn] = nc.dram_tensor(wn, (8, P, 1024), BF16, kind="ExternalInput").ap()
    for bn in ("bQ", "bPQ", "bK", "bKP", "bO"):
        io[bn] = nc.dram_tensor(bn, (P, 8), F32, kind="ExternalInput").ap()
    io["biasS"] = nc.dram_tensor(
        "biasS", (4, H, P, 384), BF16, kind="ExternalInput"
    ).ap()
    io["weiB"] = nc.dram_tensor(
        "weiB", (H, 4, P, KV), F32, kind="ExternalOutput"
    ).ap()
    io["foutT"] = nc.dram_tensor(
        "foutT", (8, P, ROWS), F32, kind="ExternalOutput"
    ).ap()

    with tile.TileContext(nc) as tc:
        _emit(nc, tc, io)
    nc.compile()
    _CACHE["nc"] = nc
    _CACHE["io"] = io
    return nc, io


def _host_prep(x, pos_emb, Wq, bq, Wk, bk, Wv, bv, Wpq, bpq, Wpk, bpk, Wo, bo):
    """Build per-core input maps (host-side data formatting only)."""
    x = np.asarray(x, np.float32)
    pos_emb = np.asarray(pos_emb, np.float32)
    weights = {
        "Wq": np.asarray(Wq, np.float32),
        "Wpq": np.asarray(Wpq, np.float32),
        "Wk": np.asarray(Wk, np.float32),
        "Wpk": np.asarray(Wpk, np.float32),
        "Wv": np.asarray(Wv, np.float32),
        "Wo": np.asarray(Wo, np.float32),
    }
    weights["Wk8"] = weights["Wk"] * 0.125

    shared = {}
    for wn, wv_ in weights.items():
        if wn == "Wk":
            continue  # only Wk8 = Wk/8 is shipped
        shared[wn] = np.ascontiguousarray(
            wv_.astype(NPBF16).reshape(8, P, 1024)
        )

    def bcol(v):  # (1024,) -> (128, 8) with column t = v[128t:128(t+1)]
        return np.ascontiguousarray(
            np.asarray(v, np.float32).reshape(8, P).T
        ).astype(np.float32)

    shared["bQ"] = bcol(np.asarray(bq, np.float32) * S192)
    shared["bPQ"] = bcol(np.asarray(bpq, np.float32) * 8.0 * S192)
    shared["bK"] = bcol(np.asarray(bk, np.float32) * 0.125)
    shared["bKP"] = bcol(np.asarray(bpk, np.float32))
    shared["bO"] = bcol(
        np.asarray(bo, np.float64)
        + np.asarray(bv, np.float64) @ np.asarray(Wo, np.float64)
    )

    # additive bias strips: rel = r - s + 256 ; allowed iff 0 <= rel <= 256
    slopes = 2.0 ** (-(8.0 / H) * np.arange(1, H + 1, dtype=np.float64))
    r = np.arange(P)[:, None]
    s = np.arange(384)[None, :]
    rel = r - s + 256
    allowed = (rel >= 0) & (rel <= 256)
    base = np.where(
        allowed[None, :, :],
        -slopes[:, None, None] * rel[None, :, :],
        NEG,
    ).astype(np.float32)  # (H, 128, 384)
    bias_int = np.broadcast_to(base, (4, H, P, 384)).copy()  # interior cores
    bias_edge = bias_int.copy()  # half==0: mask phantom j<0 (qt*128 + s < 256)
    for qt in range(4):
        ph = (qt * P + np.arange(384)) < 256
        if ph.any():
            bias_edge[qt, :, :, ph] = NEG
    bias_int = bias_int.astype(NPBF16)
    bias_edge = bias_edge.astype(NPBF16)

    in_maps = []
    for core in range(NCORES):
        b, half = divmod(core, 2)
        m = dict(shared)
        xt = np.zeros((C, KV), NPBF16)
        pt = np.zeros((C, KV), NPBF16)
        if half == 0:
            xt[:, 256:] = x[b].T[:, 0:ROWS].astype(NPBF16)
            pt[:, 256:] = pos_emb[b].T[:, 0:ROWS].astype(NPBF16)
            m["biasS"] = bias_edge
        else:
            xt[:] = x[b].T[:, 256:1024].astype(NPBF16)
            pt[:] = pos_emb[b].T[:, 256:1024].astype(NPBF16)
            m["biasS"] = bias_int
        m["xT"] = np.ascontiguousarray(xt.reshape(8, P, KV))
        m["pT"] = np.ascontiguousarray(pt.reshape(8, P, KV))
        in_maps.append(m)
    return in_maps


def run(inputs, trace=False, **kw):
    nc, io = _build()
    in_maps = _host_prep(**inputs)
    res = bass_utils.run_bass_kernel_spmd(
        nc, in_maps, core_ids=list(range(NCORES)), trace=trace, **kw
    )
    out = np.empty((B, T, C), np.float32)
    wei = np.zeros((B, H, T, T), np.float32)
    for core in range(NCORES):
        b, half = divmod(core, 2)
        r = res.results[core]
        foutT = np.asarray(r["foutT"], np.float32).reshape(C, ROWS)
        weiB = np.asarray(r["weiB"], np.float32).reshape(H, ROWS, KV)
        out[b, half * ROWS : (half + 1) * ROWS, :] = foutT.T
        if half == 0:
            wei[b, :, 0:ROWS, 0:ROWS] = weiB[:, :, 256:KV]
        else:
            wei[b, :, ROWS:T, 256:T] = weiB
    return (out, wei), res


def kernel(**inputs):
    outs, _ = run(inputs, trace=False)
    return outs
